# revision 9
# baseline (speedup 1.0000x reference)
"""DiVeQ vector-quantizer forward kernel for Trainium2 (Bass/Tile).

Problem: z [8, 64, 96, 96] f32, codebook [2048, 64] f32, ema_probs [2048] f32.
Returns (quantized, hard_spatial, indices_spatial, likelihoods,
         log_likelihoods, perplexity) matching reference.py.

Sharding: data-parallel over the batch axis N=8 -> one image per NeuronCore.
Each core sees z[n] in its native [C=64, H*W=9216] layout (channels on
partitions), so no input transpose is needed.

Per-core algorithm, tiled over 72 token-tiles of 128 tokens:
  - m[t, k] = 2*z_t.c_k - ||c_k||^2  (argmax m == argmin ||z-c||^2) via one
    fp32 PE matmul with an augmented contraction row of ones against
    mT = [2*codebook^T ; -||c||^2].
  - running max of m along k via DVE tensor_tensor_scan (op=max).
  - first-occurrence argmax index = sum_k sign(mmax - rm_k) via one ScalarE
    Sign pass with accumulate (exact: positions strictly before the first
    max contribute 1, later ones 0).
  - hard codes + probs gathered with one indirect DMA per tile from a
    prepared DRAM table [2048, 65] = [codebook | ema_probs].
  - PE transpose puts hard back into channel-major layout for the output.
"""

import json

import numpy as np

_CACHE = {}


def _split_multi_waits(js_bytes):
    """The walrus build in this container accepts at most ONE embedded sync
    wait per instruction ("Too many sync wait commands" otherwise), while
    Tile freely emits several.  Rewrite the BIR JSON: extra waits become
    standalone single-wait EventSemaphore instructions placed immediately
    before the owner on the same engine (engine program order preserves
    semantics)."""
    js = json.loads(js_bytes)
    ctr = 0
    for fn in js.get("functions", []):
        for blk in fn.get("blocks", []):
            newlist = []
            for ins in blk["instructions"]:
                si = ins.get("sync_info")
                waits = (si or {}).get("on_wait") or []
                if len(waits) > 1:
                    for w in waits[:-1]:
                        ctr += 1
                        newlist.append(
                            {
                                "debug": ins.get("debug", 0),
                                "engine": ins["engine"],
                                "ins": [],
                                "outs": [],
                                "name": f"{ins['name']}-w{ctr}",
                                "opcode": "EventSemaphore",
                                "sync_info": {"on_update": [], "on_wait": [w]},
                            }
                        )
                    si["on_wait"] = [waits[-1]]
                newlist.append(ins)
            blk["instructions"] = newlist
    return json.dumps(js).encode()


def _install_wait_split_patch():
    import concourse.bass as bass

    if getattr(bass.Bass, "_wait_split_patched", False):
        return
    orig = bass.Bass.to_json_bytes

    def patched(self):
        return _split_multi_waits(orig(self))

    bass.Bass.to_json_bytes = patched
    bass.Bass._wait_split_patched = True

N, C, H, W = 8, 64, 96, 96
HW = H * W            # 9216 tokens per image / core
K = 2048              # codebook size
TT = 128              # tokens per tile
NT = HW // TT         # 72 tiles
NEG_BIG = -3.0e38


def _build():
    _install_wait_split_patch()
    import concourse.bass as bass
    import concourse.mybir as mybir
    import concourse.tile as tile
    from concourse.masks import make_identity

    f32 = mybir.dt.float32
    i32 = mybir.dt.int32

    nc = bass.Bass()

    z_local = nc.dram_tensor("z_local", [C, HW], f32, kind="ExternalInput")
    codebook = nc.dram_tensor("codebook", [K, C], f32, kind="ExternalInput")
    ema_probs = nc.dram_tensor("ema_probs", [K], f32, kind="ExternalInput")

    q_out = nc.dram_tensor("q_out", [C, HW], f32, kind="ExternalOutput")
    idx_out = nc.dram_tensor("idx_out", [HW], i32, kind="ExternalOutput")
    prob_out = nc.dram_tensor("prob_out", [HW], f32, kind="ExternalOutput")
    logp_out = nc.dram_tensor("logp_out", [HW], f32, kind="ExternalOutput")
    ppl_out = nc.dram_tensor("ppl_out", [1, 1], f32, kind="ExternalOutput")

    AF = mybir.ActivationFunctionType

    with tile.TileContext(nc) as tc:
        with (
            tc.tile_pool(name="const", bufs=1) as const_pool,
            tc.tile_pool(name="cb", bufs=2) as cb_pool,
            tc.tile_pool(name="rm", bufs=2) as rm_pool,
            tc.tile_pool(name="small", bufs=3) as small_pool,
            tc.tile_pool(name="dist", bufs=3, space="PSUM") as dist_pool,
            tc.tile_pool(name="tpose", bufs=2, space="PSUM") as tpose_pool,
            tc.tile_pool(name="dram", bufs=1, space="DRAM") as dram_pool,
        ):
            # ---------------- constants / prep ----------------
            identity = const_pool.tile([128, 128], f32)
            make_identity(nc, identity[:])

            neg_big = const_pool.tile([128, 1], f32)
            nc.vector.memset(neg_big[:], NEG_BIG)

            # z with an appended row of ones: [65, HW]
            z_aug = const_pool.tile([C + 1, HW], f32)
            nc.sync.dma_start(out=z_aug[0:C, :], in_=z_local[:, :])
            nc.vector.memset(z_aug[C : C + 1, :], 1.0)

            # mT = [2*codebook^T ; -||c||^2]  -> [65, K]
            mT = const_pool.tile([C + 1, K], f32)
            # combined gather table [K, 65] = [codebook | ema_probs]
            combined = dram_pool.tile([K, C + 1], f32)

            for ch in range(K // 128):
                cb_chunk = cb_pool.tile([128, C], f32, tag="cbch")
                nc.sync.dma_start(
                    out=cb_chunk[:], in_=codebook[ch * 128 : (ch + 1) * 128, :]
                )
                # transpose -> [64, 128], scaled by 2 on the way out of PSUM
                ct_ps = tpose_pool.tile([C, 128], f32, tag="tp")
                nc.tensor.transpose(out=ct_ps[:], in_=cb_chunk[:], identity=identity[:])
                nc.scalar.mul(
                    mT[0:C, ch * 128 : (ch + 1) * 128], ct_ps[:], 2.0
                )

                # combined table row block: [codebook | probs]
                comb_sb = cb_pool.tile([128, C + 1], f32, tag="combsb")
                nc.vector.tensor_copy(comb_sb[:, 0:C], cb_chunk[:])
                nc.sync.dma_start(
                    out=comb_sb[:, C : C + 1],
                    in_=ema_probs[ch * 128 : (ch + 1) * 128].rearrange(
                        "(p o) -> p o", o=1
                    ),
                )
                nc.sync.dma_start(
                    out=combined[ch * 128 : (ch + 1) * 128, :], in_=comb_sb[:]
                )

            # csqT = (2c)^2 ^T ; row of -||c||^2 = -0.25 * ones @ csqT
            csqT = const_pool.tile([C, K], f32)
            nc.scalar.activation(csqT[:], mT[0:C, :], AF.Square)
            ones64 = const_pool.tile([C, 1], f32)
            nc.vector.memset(ones64[:], 1.0)
            for hh in range(2):
                cn_ps = dist_pool.tile([1, 1024], f32, tag="dist")
                for q in range(2):
                    nc.tensor.matmul(
                        out=cn_ps[:, q * 512 : (q + 1) * 512],
                        lhsT=ones64[:],
                        rhs=csqT[:, hh * 1024 + q * 512 : hh * 1024 + (q + 1) * 512],
                        start=True,
                        stop=True,
                    )
                nc.scalar.mul(
                    mT[C : C + 1, hh * 1024 : (hh + 1) * 1024], cn_ps[:], -0.25
                )

            # staging for small per-tile outputs: row j = [prob | logp | idxf]
            stage3 = const_pool.tile([NT, 3 * TT], f32)

            # collapse the many prep producers into one sync point so
            # main-loop instructions don't exceed the per-inst wait limit
            tc.strict_bb_all_engine_barrier()

            # ---------------- main loop over token tiles ----------------
            for j in range(NT):
                tok = slice(j * TT, (j + 1) * TT)
                lhsT = z_aug[:, tok]

                rm = rm_pool.tile([TT, K], f32, tag="rm")
                for hh in range(2):
                    dist = dist_pool.tile([TT, 1024], f32, tag="dist")
                    for q in range(2):
                        nc.tensor.matmul(
                            out=dist[:, q * 512 : (q + 1) * 512],
                            lhsT=lhsT,
                            rhs=mT[:, hh * 1024 + q * 512 : hh * 1024 + (q + 1) * 512],
                            start=True,
                            stop=True,
                        )
                    # running max along k
                    nc.vector.tensor_tensor_scan(
                        out=rm[:, hh * 1024 : (hh + 1) * 1024],
                        data0=dist[:],
                        data1=neg_big[:].to_broadcast([TT, 1024]),
                        initial=(
                            NEG_BIG if hh == 0 else rm[:, hh * 1024 - 1 : hh * 1024]
                        ),
                        op0=mybir.AluOpType.max,
                        op1=mybir.AluOpType.max,
                    )

                # idx = #positions strictly before the first max
                idxf = small_pool.tile([TT, 1], f32, tag="idxf")
                sgn_dump = rm_pool.tile([TT, K], f32, tag="sgn")
                nc.scalar.activation(
                    sgn_dump[:],
                    rm[:],
                    AF.Sign,
                    bias=rm[:, K - 1 : K],
                    scale=-1.0,
                    accum_out=idxf[:],
                )

                idx_i = small_pool.tile([TT, 1], i32, tag="idxi")
                nc.vector.tensor_copy(idx_i[:], idxf[:])

                # gather [hard | prob] rows from the combined table
                gath = small_pool.tile([TT, C + 3], f32, tag="gath")
                nc.gpsimd.indirect_dma_start(
                    out=gath[:, 0 : C + 1],
                    out_offset=None,
                    in_=combined[:],
                    in_offset=bass.IndirectOffsetOnAxis(ap=idx_i[:, 0:1], axis=0),
                )
                # col 65 = log(prob), col 66 = idx (as f32)
                nc.scalar.activation(gath[:, C + 1 : C + 2], gath[:, C : C + 1], AF.Ln)
                nc.vector.tensor_copy(gath[:, C + 2 : C + 3], idxf[:])

                # transpose [128, 67] -> [67, 128]
                tp = tpose_pool.tile([C + 3, TT], f32, tag="tp")
                nc.tensor.transpose(
                    out=tp[:], in_=gath[:], identity=identity[:]
                )

                hardT = cb_pool.tile([C + 3, TT], f32, tag="hardT")
                nc.scalar.copy(hardT[:], tp[:])
                nc.sync.dma_start(out=q_out[:, tok], in_=hardT[0:C, :])

                # [prob row | logp row | idxf row] -> stage3 row j
                nc.sync.dma_start(
                    out=stage3[j : j + 1, :], in_=hardT[C : C + 3, :]
                )

            # ---------------- tail: small outputs ----------------
            tc.strict_bb_all_engine_barrier()

            idx_stage = const_pool.tile([NT, TT], i32)
            nc.vector.tensor_copy(idx_stage[:], stage3[:, 2 * TT : 3 * TT])

            nc.sync.dma_start(
                out=prob_out.rearrange("(j t) -> j t", t=TT),
                in_=stage3[:, 0:TT],
            )
            nc.sync.dma_start(
                out=logp_out.rearrange("(j t) -> j t", t=TT),
                in_=stage3[:, TT : 2 * TT],
            )
            nc.sync.dma_start(
                out=idx_out.rearrange("(j t) -> j t", t=TT), in_=idx_stage[:]
            )

            # ---------------- perplexity (identical on every core) --------
            ep = const_pool.tile([128, K // 128], f32)
            nc.sync.dma_start(
                out=ep[:], in_=ema_probs[:].rearrange("(p c) -> p c", c=K // 128)
            )
            lp = const_pool.tile([128, K // 128], f32)
            nc.scalar.activation(lp[:], ep[:], AF.Ln)
            plp_dump = const_pool.tile([128, K // 128], f32)
            entp = const_pool.tile([128, 1], f32)
            nc.vector.tensor_tensor(
                out=plp_dump[:], in0=ep[:], in1=lp[:], op=mybir.AluOpType.mult
            )
            nc.vector.tensor_reduce(
                out=entp[:],
                in_=plp_dump[:],
                axis=mybir.AxisListType.X,
                op=mybir.AluOpType.add,
            )
            ones128 = const_pool.tile([128, 1], f32)
            nc.vector.memset(ones128[:], 1.0)
            ent_ps = tpose_pool.tile([1, 1], f32, tag="tp")
            nc.tensor.matmul(
                out=ent_ps[:], lhsT=entp[:], rhs=ones128[:], start=True, stop=True
            )
            ppl_sb = const_pool.tile([1, 1], f32)
            nc.scalar.activation(ppl_sb[:], ent_ps[:], AF.Exp, scale=-1.0)
            nc.sync.dma_start(out=ppl_out[:], in_=ppl_sb[:])

    return nc


def _get_nc():
    if "nc" not in _CACHE:
        _CACHE["nc"] = _build()
    return _CACHE["nc"]


TRACE = False
LAST_RESULT = None


def kernel(z, codebook, ema_probs, **_ignored):
    global LAST_RESULT
    from concourse.bass_utils import run_bass_kernel_spmd

    z = np.ascontiguousarray(np.asarray(z, dtype=np.float32))
    codebook = np.ascontiguousarray(np.asarray(codebook, dtype=np.float32))
    ema_probs = np.ascontiguousarray(np.asarray(ema_probs, dtype=np.float32))

    nc = _get_nc()
    in_maps = [
        {
            "z_local": z[n].reshape(C, HW),
            "codebook": codebook,
            "ema_probs": ema_probs,
        }
        for n in range(N)
    ]
    LAST_RESULT = run_bass_kernel_spmd(
        nc, in_maps, core_ids=list(range(N)), trace=TRACE
    )
    res = LAST_RESULT.results

    quantized = np.stack([res[n]["q_out"].reshape(C, H, W) for n in range(N)])
    indices = np.stack([res[n]["idx_out"].reshape(H, W) for n in range(N)])
    probs = np.stack([res[n]["prob_out"].reshape(1, H, W) for n in range(N)])
    logp = np.stack([res[n]["logp_out"].reshape(1, H, W) for n in range(N)])
    ppl = np.float32(res[0]["ppl_out"].reshape(())[()])

    return (quantized, quantized.copy(), indices, probs, logp, ppl)


# revision 14
# speedup vs baseline: 1.1873x; 1.1873x over previous
"""DiVeQ vector-quantizer forward kernel for Trainium2 (Bass/Tile).

Problem: z [8, 64, 96, 96] f32, codebook [2048, 64] f32, ema_probs [2048] f32.
Returns (quantized, hard_spatial, indices_spatial, likelihoods,
         log_likelihoods, perplexity) matching reference.py.

Sharding: data-parallel over the batch axis N=8 -> one image per NeuronCore.
Each core sees z[n] in its native [C=64, H*W=9216] layout (channels on
partitions), so no input transpose is needed.

Per-core algorithm, tiled over 72 token-tiles of 128 tokens:
  - m[t, k] = 2*z_t.c_k - ||c_k||^2  (argmax m == argmin ||z-c||^2) via one
    fp32 PE matmul with an augmented contraction row of ones against
    mT = [2*codebook^T ; -||c||^2].
  - running max of m along k via DVE tensor_tensor_scan (op=max).
  - first-occurrence argmax index = sum_k sign(mmax - rm_k) via one ScalarE
    Sign pass with accumulate (exact: positions strictly before the first
    max contribute 1, later ones 0).
  - hard codes + probs gathered with one indirect DMA per tile from a
    prepared DRAM table [2048, 65] = [codebook | ema_probs].
  - PE transpose puts hard back into channel-major layout for the output.
"""

import json

import numpy as np

_CACHE = {}


def _split_multi_waits(js_bytes):
    """The walrus build in this container accepts at most ONE embedded sync
    wait per instruction ("Too many sync wait commands" otherwise), while
    Tile freely emits several.  Rewrite the BIR JSON: extra waits become
    standalone single-wait EventSemaphore instructions placed immediately
    before the owner on the same engine (engine program order preserves
    semantics)."""
    js = json.loads(js_bytes)
    ctr = 0
    for fn in js.get("functions", []):
        for blk in fn.get("blocks", []):
            newlist = []
            for ins in blk["instructions"]:
                si = ins.get("sync_info")
                waits = (si or {}).get("on_wait") or []
                if len(waits) > 1:
                    for w in waits[:-1]:
                        ctr += 1
                        newlist.append(
                            {
                                "debug": ins.get("debug", 0),
                                "engine": ins["engine"],
                                "ins": [],
                                "outs": [],
                                "name": f"{ins['name']}-w{ctr}",
                                "opcode": "EventSemaphore",
                                "sync_info": {"on_update": [], "on_wait": [w]},
                            }
                        )
                    si["on_wait"] = [waits[-1]]
                newlist.append(ins)
            blk["instructions"] = newlist
    return json.dumps(js).encode()


def _install_wait_split_patch():
    import concourse.bass as bass

    if getattr(bass.Bass, "_wait_split_patched", False):
        return
    orig = bass.Bass.to_json_bytes

    def patched(self):
        return _split_multi_waits(orig(self))

    bass.Bass.to_json_bytes = patched
    bass.Bass._wait_split_patched = True

N, C, H, W = 8, 64, 96, 96
HW = H * W            # 9216 tokens per image / core
K = 2048              # codebook size
TT = 128              # tokens per tile
NT = HW // TT         # 72 tiles
NEG_BIG = -3.0e38


def _build():
    _install_wait_split_patch()
    import concourse.bass as bass
    import concourse.mybir as mybir
    import concourse.tile as tile
    from concourse.masks import make_identity

    f32 = mybir.dt.float32
    bf16 = mybir.dt.bfloat16
    i32 = mybir.dt.int32

    nc = bass.Bass()

    z_local = nc.dram_tensor("z_local", [C, HW], f32, kind="ExternalInput")
    codebook = nc.dram_tensor("codebook", [K, C], f32, kind="ExternalInput")
    ema_probs = nc.dram_tensor("ema_probs", [K], f32, kind="ExternalInput")

    q_out = nc.dram_tensor("q_out", [C, HW], f32, kind="ExternalOutput")
    idx_out = nc.dram_tensor("idx_out", [HW], i32, kind="ExternalOutput")
    prob_out = nc.dram_tensor("prob_out", [HW], f32, kind="ExternalOutput")
    logp_out = nc.dram_tensor("logp_out", [HW], f32, kind="ExternalOutput")
    ppl_out = nc.dram_tensor("ppl_out", [1, 1], f32, kind="ExternalOutput")

    AF = mybir.ActivationFunctionType
    CA = C + 2  # augmented contraction: z rows + two ones rows for -||c||^2

    with tile.TileContext(nc) as tc:
        with (
            tc.tile_pool(name="const", bufs=1) as const_pool,
            tc.tile_pool(name="cb", bufs=2) as cb_pool,
            tc.tile_pool(name="rm", bufs=2) as rm_pool,
            tc.tile_pool(name="small", bufs=3) as small_pool,
            tc.tile_pool(name="dist", bufs=3, space="PSUM") as dist_pool,
            tc.tile_pool(name="tpose", bufs=2, space="PSUM") as tpose_pool,
            tc.tile_pool(name="dram", bufs=1, space="DRAM") as dram_pool,
        ):
            # ---------------- constants / prep ----------------
            identity = const_pool.tile([128, 128], f32)
            make_identity(nc, identity[:])

            neg_big = const_pool.tile([128, 1], f32)
            nc.vector.memset(neg_big[:], NEG_BIG)

            # z split into bf16 hi/lo with two appended ones rows: [66, HW]
            # m[t,k] = 2 z_t.c_k - ||c_k||^2 computed as three bf16 matmuls:
            #   zh.mh + zh.ml + zl.mh   (zl rows 64/65 are 0 so mh's cnorm
            #   rows are only added once; ml carries the other cnorm chunks)
            z_f32 = const_pool.tile([C, HW], f32)
            nc.sync.dma_start(out=z_f32[:], in_=z_local[:, :])
            zh_aug = const_pool.tile([CA, HW], bf16)
            zl_aug = const_pool.tile([CA, HW], bf16)
            nc.vector.tensor_copy(zh_aug[0:C, :], z_f32[:])
            nc.vector.tensor_tensor(
                out=zl_aug[0:C, :],
                in0=z_f32[:],
                in1=zh_aug[0:C, :],
                op=mybir.AluOpType.subtract,
            )
            nc.vector.memset(zh_aug[C : C + 2, :], 1.0)
            nc.vector.memset(zl_aug[C : C + 2, :], 0.0)

            # moving matrices mh/ml [66, K] bf16:
            #  mh rows 0-63 = bf16(2 c^T)          ml rows 0-63 = residual
            #  mh rows 64/65 = -cn_a/-cn_c         ml rows 64/65 = -cn_b/-cn_d
            # where cn_a..d are a 4-chunk bf16 decomposition of ||c||^2.
            mh = const_pool.tile([CA, K], bf16)
            ml = const_pool.tile([CA, K], bf16)
            c2T = const_pool.tile([C, K], f32)
            # combined gather table [K, 65] = [codebook | ema_probs]
            combined = dram_pool.tile([K, C + 1], f32)

            for ch in range(K // 128):
                cb_chunk = cb_pool.tile([128, C], f32, tag="cbch")
                nc.sync.dma_start(
                    out=cb_chunk[:], in_=codebook[ch * 128 : (ch + 1) * 128, :]
                )
                # transpose -> [64, 128], scaled by 2 on the way out of PSUM
                ct_ps = tpose_pool.tile([C, 128], f32, tag="tp")
                nc.tensor.transpose(out=ct_ps[:], in_=cb_chunk[:], identity=identity[:])
                nc.scalar.mul(
                    c2T[:, ch * 128 : (ch + 1) * 128], ct_ps[:], 2.0
                )

                # combined table row block: [codebook | probs]
                comb_sb = cb_pool.tile([128, C + 1], f32, tag="combsb")
                nc.vector.tensor_copy(comb_sb[:, 0:C], cb_chunk[:])
                nc.sync.dma_start(
                    out=comb_sb[:, C : C + 1],
                    in_=ema_probs[ch * 128 : (ch + 1) * 128].rearrange(
                        "(p o) -> p o", o=1
                    ),
                )
                nc.sync.dma_start(
                    out=combined[ch * 128 : (ch + 1) * 128, :], in_=comb_sb[:]
                )

            nc.vector.tensor_copy(mh[0:C, :], c2T[:])
            nc.vector.tensor_tensor(
                out=ml[0:C, :],
                in0=c2T[:],
                in1=mh[0:C, :],
                op=mybir.AluOpType.subtract,
            )

            # csqT = (2c)^2 ^T ; ||c||^2 = 0.25 * ones @ csqT
            csqT = const_pool.tile([C, K], f32)
            nc.scalar.activation(csqT[:], c2T[:], AF.Square)
            ones64 = const_pool.tile([C, 1], f32)
            nc.vector.memset(ones64[:], 1.0)
            cnorm = const_pool.tile([1, K], f32)
            for hh in range(2):
                cn_ps = dist_pool.tile([1, 1024], f32, tag="dist")
                for q in range(2):
                    nc.tensor.matmul(
                        out=cn_ps[:, q * 512 : (q + 1) * 512],
                        lhsT=ones64[:],
                        rhs=csqT[:, hh * 1024 + q * 512 : hh * 1024 + (q + 1) * 512],
                        start=True,
                        stop=True,
                    )
                nc.scalar.mul(
                    cnorm[:, hh * 1024 : (hh + 1) * 1024], cn_ps[:], 0.25
                )
            # 4-chunk bf16 split of -cnorm, staged at partition 0 then DMA'd
            # into mh/ml rows C/C+1 (TT ops need equal base partitions)
            cn_rem = const_pool.tile([1, K], f32)
            cn_negs = [
                const_pool.tile([1, K], bf16, name=f"cn_neg{i}") for i in range(4)
            ]
            src = cnorm
            for i in range(4):
                nc.vector.tensor_scalar_mul(cn_negs[i][:], src[:], -1.0)
                if i < 3:
                    nc.vector.tensor_tensor(
                        out=cn_rem[:], in0=src[:], in1=cn_negs[i][:],
                        op=mybir.AluOpType.add,
                    )
                    src = cn_rem
            nc.sync.dma_start(out=mh[C : C + 1, :], in_=cn_negs[0][:])
            nc.sync.dma_start(out=ml[C : C + 1, :], in_=cn_negs[1][:])
            nc.sync.dma_start(out=mh[C + 1 : C + 2, :], in_=cn_negs[2][:])
            nc.sync.dma_start(out=ml[C + 1 : C + 2, :], in_=cn_negs[3][:])

            # staging for small per-tile outputs: row j = [prob | logp | idxf]
            stage3 = const_pool.tile([NT, 3 * TT], f32)

            # collapse the many prep producers into one sync point so
            # main-loop instructions don't exceed the per-inst wait limit
            tc.strict_bb_all_engine_barrier()

            # ---------------- main loop over token tiles ----------------
            for j in range(NT):
                tok = slice(j * TT, (j + 1) * TT)

                rm = rm_pool.tile([TT, K], f32, tag="rm")
                dists = [
                    dist_pool.tile([TT, 1024], f32, tag="dist", name=f"dist{j}_{h}")
                    for h in range(2)
                ]
                # zh terms first (shared stationary), then the zl term
                for term, (lhsT, rhs, st, sp) in enumerate(
                    [
                        (zh_aug, mh, True, False),
                        (zh_aug, ml, False, False),
                        (zl_aug, mh, False, True),
                    ]
                ):
                    for hh in range(2):
                        for q in range(2):
                            ks = slice(
                                hh * 1024 + q * 512, hh * 1024 + (q + 1) * 512
                            )
                            nc.tensor.matmul(
                                out=dists[hh][:, q * 512 : (q + 1) * 512],
                                lhsT=lhsT[:, tok],
                                rhs=rhs[:, ks],
                                start=st,
                                stop=sp,
                            )
                for hh in range(2):
                    # running max along k
                    nc.vector.tensor_tensor_scan(
                        out=rm[:, hh * 1024 : (hh + 1) * 1024],
                        data0=dists[hh][:],
                        data1=neg_big[:].to_broadcast([TT, 1024]),
                        initial=(
                            NEG_BIG if hh == 0 else rm[:, hh * 1024 - 1 : hh * 1024]
                        ),
                        op0=mybir.AluOpType.max,
                        op1=mybir.AluOpType.max,
                    )

                # idx = #positions strictly before the first max
                idxf = small_pool.tile([TT, 1], f32, tag="idxf")
                sgn_dump = rm_pool.tile([TT, K], f32, tag="sgn")
                nc.scalar.activation(
                    sgn_dump[:],
                    rm[:],
                    AF.Sign,
                    bias=rm[:, K - 1 : K],
                    scale=-1.0,
                    accum_out=idxf[:],
                )

                idx_i = small_pool.tile([TT, 1], i32, tag="idxi")
                nc.vector.tensor_copy(idx_i[:], idxf[:])

                # gather [hard | prob] rows from the combined table
                gath = small_pool.tile([TT, C + 3], f32, tag="gath")
                nc.gpsimd.indirect_dma_start(
                    out=gath[:, 0 : C + 1],
                    out_offset=None,
                    in_=combined[:],
                    in_offset=bass.IndirectOffsetOnAxis(ap=idx_i[:, 0:1], axis=0),
                )
                # col 65 = log(prob), col 66 = idx (as f32)
                nc.scalar.activation(gath[:, C + 1 : C + 2], gath[:, C : C + 1], AF.Ln)
                nc.vector.tensor_copy(gath[:, C + 2 : C + 3], idxf[:])

                # transpose [128, 67] -> [67, 128]
                tp = tpose_pool.tile([C + 3, TT], f32, tag="tp")
                nc.tensor.transpose(
                    out=tp[:], in_=gath[:], identity=identity[:]
                )

                hardT = cb_pool.tile([C + 3, TT], f32, tag="hardT")
                nc.scalar.copy(hardT[:], tp[:])
                nc.sync.dma_start(out=q_out[:, tok], in_=hardT[0:C, :])

                # [prob row | logp row | idxf row] -> stage3 row j
                nc.sync.dma_start(
                    out=stage3[j : j + 1, :], in_=hardT[C : C + 3, :]
                )

            # ---------------- tail: small outputs ----------------
            tc.strict_bb_all_engine_barrier()

            idx_stage = const_pool.tile([NT, TT], i32)
            nc.vector.tensor_copy(idx_stage[:], stage3[:, 2 * TT : 3 * TT])

            nc.sync.dma_start(
                out=prob_out.rearrange("(j t) -> j t", t=TT),
                in_=stage3[:, 0:TT],
            )
            nc.sync.dma_start(
                out=logp_out.rearrange("(j t) -> j t", t=TT),
                in_=stage3[:, TT : 2 * TT],
            )
            nc.sync.dma_start(
                out=idx_out.rearrange("(j t) -> j t", t=TT), in_=idx_stage[:]
            )

            # ---------------- perplexity (identical on every core) --------
            ep = const_pool.tile([128, K // 128], f32)
            nc.sync.dma_start(
                out=ep[:], in_=ema_probs[:].rearrange("(p c) -> p c", c=K // 128)
            )
            lp = const_pool.tile([128, K // 128], f32)
            nc.scalar.activation(lp[:], ep[:], AF.Ln)
            plp_dump = const_pool.tile([128, K // 128], f32)
            entp = const_pool.tile([128, 1], f32)
            nc.vector.tensor_tensor(
                out=plp_dump[:], in0=ep[:], in1=lp[:], op=mybir.AluOpType.mult
            )
            nc.vector.tensor_reduce(
                out=entp[:],
                in_=plp_dump[:],
                axis=mybir.AxisListType.X,
                op=mybir.AluOpType.add,
            )
            ones128 = const_pool.tile([128, 1], f32)
            nc.vector.memset(ones128[:], 1.0)
            ent_ps = tpose_pool.tile([1, 1], f32, tag="tp")
            nc.tensor.matmul(
                out=ent_ps[:], lhsT=entp[:], rhs=ones128[:], start=True, stop=True
            )
            ppl_sb = const_pool.tile([1, 1], f32)
            nc.scalar.activation(ppl_sb[:], ent_ps[:], AF.Exp, scale=-1.0)
            nc.sync.dma_start(out=ppl_out[:], in_=ppl_sb[:])

    return nc


def _get_nc():
    if "nc" not in _CACHE:
        _CACHE["nc"] = _build()
    return _CACHE["nc"]


TRACE = False
LAST_RESULT = None


def kernel(z, codebook, ema_probs, **_ignored):
    global LAST_RESULT
    from concourse.bass_utils import run_bass_kernel_spmd

    z = np.ascontiguousarray(np.asarray(z, dtype=np.float32))
    codebook = np.ascontiguousarray(np.asarray(codebook, dtype=np.float32))
    ema_probs = np.ascontiguousarray(np.asarray(ema_probs, dtype=np.float32))

    nc = _get_nc()
    in_maps = [
        {
            "z_local": z[n].reshape(C, HW),
            "codebook": codebook,
            "ema_probs": ema_probs,
        }
        for n in range(N)
    ]
    LAST_RESULT = run_bass_kernel_spmd(
        nc, in_maps, core_ids=list(range(N)), trace=TRACE
    )
    res = LAST_RESULT.results

    quantized = np.stack([res[n]["q_out"].reshape(C, H, W) for n in range(N)])
    indices = np.stack([res[n]["idx_out"].reshape(H, W) for n in range(N)])
    probs = np.stack([res[n]["prob_out"].reshape(1, H, W) for n in range(N)])
    logp = np.stack([res[n]["logp_out"].reshape(1, H, W) for n in range(N)])
    ppl = np.float32(res[0]["ppl_out"].reshape(())[()])

    return (quantized, quantized.copy(), indices, probs, logp, ppl)
